# revision 8
# baseline (speedup 1.0000x reference)
"""Trainium2 Bass kernel for dual-attention block (CAM + SAM + bottleneck).

Contract: kernel(**inputs) takes FULL unsharded inputs
  x     [8, 64, 64, 64] f32
  w_cam [64, 64], w_q [32, 64], w_k [32, 64], w_v [64, 64], w_bn [64, 128]
and returns the full [8, 64, 64, 64] f32 output.

Sharding: data-parallel over batch across 8 NeuronCores (1 image each);
weights replicated. Per-core math (c=64 channels, n=m=4096 spatial):

  CAM: xcT = x.T @ w_cam.T ; Ec = xcT.T @ xcT;
       attn_c = softmax_rows(Ec); bn1 = (wbn1 @ attn_c) @ x   (folded M1)
  SAM: q4/k4 = (w stacked 4x) @ x  -> q,k replicated on 4 partition groups
       S[m,n] = sum_c k[c,m] q[c,n]  (row-tiled K=32 matmuls, concurrent)
       E = exp(S)  (no max subtraction needed: |S| < ~6)
       acc[c,n] = sum_m W[m,c] E[m,n]  with W = [v.T | ones]
                  -> rows 0..63 unnormalized out_s, row 64 = Z (softmax denom)
  out = x + bn1 + (wbn2 @ acc[0:64]) * (1/Z)
        (per-n 1/Z broadcast to 64 partitions via a K=1 PE matmul)

Design (v2): the ScalarE (ACT) engine is the bottleneck -- 16.8M exp
evaluations at 1 elem/lane/cycle ~= 110us/core.  Everything else is
arranged to hide behind the exp stream:
  - ACT does ONLY exp; all PSUM->SBUF evacuation is on DVE.
  - 13 m-tile groups per n-block alternate between two PSUM pools
    (3-bank / 2-bank) so exp(g) overlaps S-matmuls of later groups.
  - vacc (the acc accumulator) is double-buffered; the 4.3us DVE
    reciprocal runs off the critical path on an SBUF copy of Z.
  - CAM folds into one [64,64] matmul M1T; the wvc (block 0) and ec
    (block 1) matmuls are interleaved into the group loop so the PE
    never runs long while ACT idles.  Epilogues are deferred two blocks
    and slotted into PE gaps.
  - PSUM: spoolA(3) + spoolB(2) + vacc/EC(2) + ppool(1) = 8 banks.
"""

import sys
from contextlib import ExitStack

import numpy as np

if "/opt/trn_rl_repo" not in sys.path:
    sys.path.insert(0, "/opt/trn_rl_repo")

import concourse.bass as bass
import concourse.tile as tile
from concourse import bacc, mybir
from concourse.bass_utils import run_bass_kernel_spmd

F32 = mybir.dt.float32
F32R = mybir.dt.float32r
BF16 = mybir.dt.bfloat16


def _r(ap):
    """fp32r view: full-rate PE streaming for fp32 data (N>=256)."""
    return ap.bitcast(F32R)


C = 64          # channels
HW = 4096       # 64*64 spatial
NB = 8          # number of 512-wide n blocks
BLK = 512
MT = 32         # m tiles of 128

# 13 m-tile groups per n-block, alternating between spoolA (3 banks) and
# spoolB (2 banks): sizes 3,2,3,2,...,3,2,2 covering 32 m-tiles.
GROUPS = []
_base = 0
for _g in range(13):
    _sz = 2 if (_g % 2 == 1 or _g == 12) else 3
    GROUPS.append((_base, _sz, _g % 2 == 0))  # (base, size, use_pool_A)
    _base += _sz
assert _base == MT, _base

Exp = mybir.ActivationFunctionType.Exp


def _build_kernel(ctx: ExitStack, tc: tile.TileContext, io: dict):
    nc = tc.nc
    x_d = io["x"]
    out_d = io["out"]

    consts = ctx.enter_context(tc.tile_pool(name="consts", bufs=1))
    bigs = ctx.enter_context(tc.tile_pool(name="bigs", bufs=1))
    epool = ctx.enter_context(tc.tile_pool(name="epool", bufs=3))
    campool = ctx.enter_context(tc.tile_pool(name="campool", bufs=1))
    sampool = ctx.enter_context(tc.tile_pool(name="sampool", bufs=2))
    spoolA = ctx.enter_context(
        tc.tile_pool(name="spoolA", bufs=1, space=bass.MemorySpace.PSUM)
    )
    spoolB = ctx.enter_context(
        tc.tile_pool(name="spoolB", bufs=1, space=bass.MemorySpace.PSUM)
    )
    vpool = ctx.enter_context(
        tc.tile_pool(name="vpool", bufs=2, space=bass.MemorySpace.PSUM)
    )
    ppool = ctx.enter_context(
        tc.tile_pool(name="ppool", bufs=1, space=bass.MemorySpace.PSUM)
    )

    # ---- constants ----
    wq4T = consts.tile([C, 128], BF16)    # (w_q stacked 4x).T
    wk4T = consts.tile([C, 128], BF16)
    wvc = consts.tile([C, 128], F32)      # [v.T | w_cam.T]
    wbn1T = consts.tile([C, C], F32)
    wbn2T = consts.tile([C, C], BF16)
    ones_r = consts.tile([128, C], BF16)  # row 64 holds ones[1, 64]
    zb = consts.tile([128, 1], F32)
    dummy = consts.tile([128, 1], F32)

    nc.sync.dma_start(wq4T[:], io["wq4T"][:])
    nc.sync.dma_start(wk4T[:], io["wk4T"][:])
    nc.sync.dma_start(wvc[:], io["wvc"][:])
    nc.sync.dma_start(wbn1T[:], io["wbn1T"][:])
    nc.sync.dma_start(wbn2T[:], io["wbn2T"][:])
    nc.sync.dma_start(ones_r[C : C + 1, :], io["ones64"][:])
    nc.vector.memset(zb[:], 0.0)
    # Trigger the exp ACT-table load at t~0 (overlaps the x DMA) instead of
    # in front of the first real exp.
    nc.scalar.activation(dummy[:], zb[:], Exp, bias=zb[:])

    x_sb = bigs.tile([C, HW], F32)
    nc.sync.dma_start(x_sb[:], x_d[:])

    q4 = bigs.tile([128, HW], BF16)
    k4 = bigs.tile([128, HW], BF16)
    wt = bigs.tile([128, MT * 65], BF16)   # per m-tile [vT | ones] chunks
    xct = bigs.tile([128, MT * C], F32)    # xcT, m-tile-major
    x_bf = bigs.tile([C, HW], BF16)

    # ones column of wt (wvc copies below only write cols 0..63 of each chunk)
    nc.vector.memset(
        wt[:].rearrange("p (t c) -> p t c", c=65)[:, :, 64:65], 1.0
    )

    # x in bf16 feeds the q4/k4/bn1 matmuls at full PE rate.
    nc.vector.tensor_copy(x_bf[:], x_sb[:])

    # ---- q4 / k4: replicated q,k via stacked-weight 1x1 convs ----
    # k4 first (block 0 needs all of k4 but only q4's first chunk).
    def qk_group(wT, dst, chunks, pool, width):
        ps = pool.tile([128, width], F32, tag="s", name="qkps")
        for i, cch in enumerate(chunks):
            nc.tensor.matmul(
                ps[:, i * BLK : (i + 1) * BLK],
                wT[:],
                x_bf[:, cch * BLK : (cch + 1) * BLK],
                start=True,
                stop=True,
            )
        w = len(chunks) * BLK
        lo = chunks[0] * BLK
        nc.vector.tensor_copy(dst[:, lo : lo + w], ps[:, :w])

    qk_group(wk4T, k4, [0, 1, 2], spoolA, 3 * BLK)
    qk_group(wq4T, q4, [0, 1], spoolB, 2 * BLK)
    qk_group(wk4T, k4, [3, 4, 5], spoolA, 3 * BLK)
    qk_group(wq4T, q4, [2, 3], spoolB, 2 * BLK)
    qk_group(wk4T, k4, [6, 7], spoolA, 3 * BLK)
    qk_group(wq4T, q4, [4, 5], spoolB, 2 * BLK)
    qk_group(wq4T, q4, [6, 7], spoolB, 2 * BLK)

    state = {}  # EC tile, allocated at block 1 start (vpool slot timing)

    def wvc_group(base, size):
        """xcT and WT (=[vT|ones]) production for one m-tile group."""
        ps_w = ppool.tile([128, BLK], F32, tag="p", name="wvcps")
        for j in range(size):
            m = base + j
            nc.tensor.matmul(
                ps_w[:, j * 128 : (j + 1) * 128],
                x_sb[:, m * 128 : (m + 1) * 128],
                wvc[:],
                start=True,
                stop=True,
            )
        src = ps_w[:, : size * 128].rearrange("p (j c) -> p j c", c=128)
        wt_dst = wt[:, base * 65 : (base + size) * 65].rearrange(
            "p (j c) -> p j c", c=65
        )
        nc.vector.tensor_copy(wt_dst[:, :, 0:C], src[:, :, 0:C])
        xct_dst = xct[:, base * C : (base + size) * C].rearrange(
            "p (j c) -> p j c", c=C
        )
        nc.vector.tensor_copy(xct_dst, src[:, :, C : 2 * C])

    def ec_group(base, size):
        EC = state["EC"]
        for j in range(size):
            m = base + j
            nc.tensor.matmul(
                EC[0:C, 0:C],
                xct[:, m * C : (m + 1) * C],
                xct[:, m * C : (m + 1) * C],
                start=(m == 0),
                stop=(m == MT - 1),
            )

    # ---- per-block state for split epilogues ----
    vaccs = [None] * NB
    sam = [None] * NB   # sam65 [65, BLK] f32: rows 0..63 unnorm out_s, 64 = Z
    rzs = [None] * NB   # rz [65, BLK] bf16: row 64 = 1/Z
    M1T_sb = campool.tile([C, C], BF16)

    def epilogue_a(nb):
        """At block end: evacuate vacc, start the (slow, off-path) recip."""
        aux = sampool.tile([C + 1, BLK], F32, tag="aux", name="aux")
        nc.vector.tensor_copy(aux[:], vaccs[nb][0 : C + 1, :])
        rzb = sampool.tile([C + 1, BLK], BF16, tag="rz", name="rzb")
        with nc.allow_low_precision(reason="1/Z in bf16: 0.4% on the SAM term"):
            nc.vector.reciprocal(rzb[C : C + 1, :], aux[C : C + 1, :])
        sam[nb] = aux
        rzs[nb] = rzb

    def epilogue_b(nb):
        """Broadcast 1/Z, scale, bottleneck conv, residual add, DMA out."""
        ncol = slice(nb * BLK, (nb + 1) * BLK)
        bc = ppool.tile([128, BLK], F32, tag="p", name="bc")
        nc.tensor.matmul(
            bc[0:C, :],
            ones_r[C : C + 1, 0:C],
            rzs[nb][C : C + 1, :],
            start=True,
            stop=True,
            tile_position=(C, 0),
        )
        sam_sc = sampool.tile([C, BLK], BF16, tag="sc", name="sam_sc")
        nc.vector.tensor_mul(sam_sc[:], sam[nb][0:C, :], bc[0:C, :])
        bn = ppool.tile([128, BLK], F32, tag="p", name="bn")
        nc.tensor.matmul(
            bn[0:C, :], M1T_sb[:], x_bf[:, ncol], start=True, stop=False
        )
        nc.tensor.matmul(
            bn[0:C, :], wbn2T[:], sam_sc[:], start=False, stop=True
        )
        o_t = sampool.tile([C, BLK], F32, tag="ot", name="o_t")
        nc.vector.tensor_add(o_t[:], x_sb[:, ncol], bn[0:C, :])
        nc.sync.dma_start(out_d[:, ncol], o_t[:])

    # ---- main SAM loop over 8 n-blocks ----
    for nb in range(NB):
        ncol = slice(nb * BLK, (nb + 1) * BLK)
        if nb == 1:
            # EC takes the vpool slot vacated by vacc(0)+1 rotation; its
            # last readers (CAM softmax, end of block 1) finish before
            # vacc(2) re-claims the slot at block 2.
            state["EC"] = vpool.tile([128, BLK], F32, tag="v", name="EC")
        vacc = vpool.tile([128, BLK], F32, tag="v", name="vacc")
        vaccs[nb] = vacc
        for gi, (base, size, useA) in enumerate(GROUPS):
            pool, width = (spoolA, 3 * BLK) if useA else (spoolB, 2 * BLK)
            s_t = pool.tile([128, width], F32, tag="s", name="s_t")
            for j in range(size):
                m = base + j
                nc.tensor.matmul(
                    s_t[:, j * BLK : (j + 1) * BLK],
                    k4[32 * j : 32 * j + 32, m * 128 : (m + 1) * 128],
                    q4[32 * j : 32 * j + 32, ncol],
                    start=True,
                    stop=True,
                    tile_position=(32 * j, 0),
                )
            if nb == 0:
                wvc_group(base, size)  # wt/xcT for this group (acc needs wt)
            if nb == 1:
                ec_group(base, size)   # CAM energy, spread across block 1
            w = size * BLK
            e_t = epool.tile([128, 3 * BLK], BF16, tag="e", name="e_t")
            nc.scalar.activation(e_t[:, :w], s_t[:, :w], Exp, bias=zb[:])
            for j in range(size):
                m = base + j
                nc.tensor.matmul(
                    vacc[0 : C + 1, :],
                    wt[:, m * 65 : (m + 1) * 65],
                    e_t[:, j * BLK : (j + 1) * BLK],
                    start=(m == 0),
                    stop=(m == MT - 1),
                )
            if nb >= 2 and gi == 1:
                # epilogue from two blocks ago, slotted into PE gaps
                epilogue_b(nb - 2)

        epilogue_a(nb)
        if nb == 1:
            # CAM softmax -> attn_c -> M1T = (wbn1 @ attn_c).T
            EC = state["EC"]
            negmax = campool.tile([C, 1], F32)
            nc.vector.reduce_max(
                negmax[:], EC[0:C, 0:C], axis=mybir.AxisListType.X, negate=True
            )
            exp_c = campool.tile([C, C], F32)
            nc.scalar.activation(exp_c[:], EC[0:C, 0:C], Exp, bias=negmax[:])
            sum_c = campool.tile([C, 1], F32)
            nc.vector.reduce_sum(sum_c[:], exp_c[:], axis=mybir.AxisListType.X)
            rec_c = campool.tile([C, 1], F32)
            nc.vector.reciprocal(rec_c[:], sum_c[:])
            attn_c = campool.tile([C, C], F32)
            nc.vector.tensor_scalar_mul(attn_c[:], exp_c[:], rec_c[:])
            m1ps = ppool.tile([128, BLK], F32, tag="p", name="m1ps")
            nc.tensor.matmul(
                m1ps[0:C, 0:C], attn_c[:], wbn1T[:], start=True, stop=True
            )
            nc.vector.tensor_copy(M1T_sb[:], m1ps[0:C, 0:C])
    epilogue_b(NB - 2)
    epilogue_b(NB - 1)


def build_nc():
    nc = bacc.Bacc(
        "TRN2",
        target_bir_lowering=False,
        debug=False,
        enable_asserts=False,
        num_devices=8,
    )
    io = {}
    io["x"] = nc.dram_tensor("x", [C, HW], F32, kind="ExternalInput").ap()
    io["wq4T"] = nc.dram_tensor("wq4T", [C, 128], BF16, kind="ExternalInput").ap()
    io["wk4T"] = nc.dram_tensor("wk4T", [C, 128], BF16, kind="ExternalInput").ap()
    io["wvc"] = nc.dram_tensor("wvc", [C, 128], F32, kind="ExternalInput").ap()
    io["wbn1T"] = nc.dram_tensor("wbn1T", [C, C], F32, kind="ExternalInput").ap()
    io["wbn2T"] = nc.dram_tensor("wbn2T", [C, C], BF16, kind="ExternalInput").ap()
    io["ones64"] = nc.dram_tensor("ones64", [1, C], BF16, kind="ExternalInput").ap()
    io["out"] = nc.dram_tensor("out", [C, HW], F32, kind="ExternalOutput").ap()

    with tile.TileContext(nc) as tc:
        with ExitStack() as ctx:
            _build_kernel(ctx, tc, io)
    nc.compile()
    return nc


def make_in_maps(x, w_cam, w_q, w_k, w_v, w_bn):
    import ml_dtypes

    f = lambda a: np.ascontiguousarray(np.asarray(a, dtype=np.float32))
    fb = lambda a: np.ascontiguousarray(
        np.asarray(a, dtype=np.float32).astype(ml_dtypes.bfloat16)
    )
    base = {
        "wq4T": fb(np.concatenate([np.asarray(w_q).T] * 4, axis=1)),
        "wk4T": fb(np.concatenate([np.asarray(w_k).T] * 4, axis=1)),
        "wvc": f(np.concatenate([np.asarray(w_v).T, np.asarray(w_cam).T], axis=1)),
        "wbn1T": f(np.asarray(w_bn)[:, :C].T),
        "wbn2T": fb(np.asarray(w_bn)[:, C:].T),
        "ones64": fb(np.ones((1, C))),
    }
    x = np.asarray(x)
    return [dict(base, x=f(x[b].reshape(C, HW))) for b in range(8)]


_NC_CACHE = None


def kernel(x, w_cam, w_q, w_k, w_v, w_bn):
    global _NC_CACHE
    if _NC_CACHE is None:
        _NC_CACHE = build_nc()
    nc = _NC_CACHE
    in_maps = make_in_maps(x, w_cam, w_q, w_k, w_v, w_bn)
    res = run_bass_kernel_spmd(nc, in_maps, list(range(8)))
    out = np.stack([res.results[b]["out"].reshape(C, 64, 64) for b in range(8)])
    return out.astype(np.float32)


# revision 14
# speedup vs baseline: 1.2884x; 1.2884x over previous
"""Trainium2 Bass kernel for dual-attention block (CAM + SAM + bottleneck).

Contract: kernel(**inputs) takes FULL unsharded inputs
  x     [8, 64, 64, 64] f32
  w_cam [64, 64], w_q [32, 64], w_k [32, 64], w_v [64, 64], w_bn [64, 128]
and returns the full [8, 64, 64, 64] f32 output.

Sharding: data-parallel over batch across 8 NeuronCores (1 image each);
weights replicated. Per-core math (c=64 channels, n=m=4096 spatial):

  CAM: xcT = x.T @ w_cam.T ; Ec = xcT.T @ xcT;
       attn_c = softmax_rows(Ec); bn1 = (wbn1 @ attn_c) @ x   (folded M1)
  SAM: q4/k4 = (w stacked 4x) @ x  -> q,k replicated on 4 partition groups
       S[m,n] = sum_c k[c,m] q[c,n]  (row-tiled K=32 matmuls, 4-concurrent)
       E = exp(S) in fp8-e4m3  (no max subtraction needed: |S| < ~6)
       acc[c,n] = sum_m W[m,c] E[m,n]  with W = [v.T | ones] in fp8,
                  one DoubleRow matmul per m-tile PAIR (K=256 contraction)
                  -> rows 0..63 unnormalized out_s, row 64 = Z
  out = x + bn1 + (wbn2 @ acc[0:64]) * (1/Z)
        (per-n 1/Z broadcast to 64 partitions via a K=1 PE matmul)

Design (v3): the ScalarE (ACT) engine is the bound -- 16.8M exp at
1 elem/lane/cycle ~= 110us/core (128us with per-instr overhead).  The
HAM clock gate keeps a <70%-duty PE at 1.2 GHz, so instead of fighting
for warmth the PE workload is cut (fp8 DoubleRow acc = 0.5 cyc/row;
4-way S row tiling) until even a COLD PE (~97us) hides under the exp
stream.  ACT does ONLY exp; all PSUM evacuation is on DVE; the 4.3us
DVE reciprocal runs off the critical path; epilogues are deferred two
blocks and slotted into PE gaps; wvc (block 0) and ec (block 1)
matmuls interleave into the group loop.
PSUM: spoolA(2) + spoolB(2) + vacc/EC(2) + ppool(2) = 8 banks.
"""

import sys
from contextlib import ExitStack

import numpy as np

if "/opt/trn_rl_repo" not in sys.path:
    sys.path.insert(0, "/opt/trn_rl_repo")

import concourse.bass as bass
import concourse.tile as tile
from concourse import bacc, mybir
from concourse.bass_utils import run_bass_kernel_spmd

F32 = mybir.dt.float32
BF16 = mybir.dt.bfloat16
FP8 = mybir.dt.float8e4

C = 64          # channels
HW = 4096       # 64*64 spatial
NB = 8          # number of 512-wide n blocks
BLK = 512
MT = 32         # m tiles of 128
NG = 16         # groups of 2 m-tiles per n-block
WP = 80         # wt8 per-m-tile stride (65 used; 80 for DoubleRow step%16==0)

Exp = mybir.ActivationFunctionType.Exp
DR = mybir.MatmulPerfMode.DoubleRow


def _build_kernel(ctx: ExitStack, tc: tile.TileContext, io: dict):
    nc = tc.nc
    x_d = io["x"]
    out_d = io["out"]

    consts = ctx.enter_context(tc.tile_pool(name="consts", bufs=1))
    bigs = ctx.enter_context(tc.tile_pool(name="bigs", bufs=1))
    epool = ctx.enter_context(tc.tile_pool(name="epool", bufs=3))
    campool = ctx.enter_context(tc.tile_pool(name="campool", bufs=1))
    sampool = ctx.enter_context(tc.tile_pool(name="sampool", bufs=2))
    spoolA = ctx.enter_context(
        tc.tile_pool(name="spoolA", bufs=1, space=bass.MemorySpace.PSUM)
    )
    spoolB = ctx.enter_context(
        tc.tile_pool(name="spoolB", bufs=1, space=bass.MemorySpace.PSUM)
    )
    vpool = ctx.enter_context(
        tc.tile_pool(name="vpool", bufs=2, space=bass.MemorySpace.PSUM)
    )
    ppool = ctx.enter_context(
        tc.tile_pool(name="ppool", bufs=2, space=bass.MemorySpace.PSUM)
    )

    # ---- constants ----
    wq4T = consts.tile([C, 128], BF16)    # (w_q stacked 4x).T
    wk4T = consts.tile([C, 128], BF16)
    wvc = consts.tile([C, 128], F32)      # [v.T | w_cam.T]
    wbn1T = consts.tile([C, C], F32)
    wbn2T = consts.tile([C, C], BF16)
    ones_r = consts.tile([128, C], BF16)  # row 64 holds ones[1, 64]
    zb = consts.tile([128, 1], F32)
    nlog64 = consts.tile([128, 1], F32)  # exp bias -ln64: E'=E/64 fits fp8e4 max 240 (max|S|=9.05)
    dummy = consts.tile([128, 1], F32)

    nc.sync.dma_start(wq4T[:], io["wq4T"][:])
    nc.sync.dma_start(wk4T[:], io["wk4T"][:])
    nc.sync.dma_start(wvc[:], io["wvc"][:])
    nc.sync.dma_start(wbn1T[:], io["wbn1T"][:])
    nc.sync.dma_start(wbn2T[:], io["wbn2T"][:])
    nc.sync.dma_start(ones_r[C : C + 1, :], io["ones64"][:])
    nc.vector.memset(zb[:], 0.0)
    nc.vector.memset(nlog64[:], -4.1588830833596715)
    # Trigger the exp ACT-table load at t~0 (overlaps the x DMA) instead of
    # in front of the first real exp.
    nc.scalar.activation(dummy[:], zb[:], Exp, bias=zb[:])

    x_sb = bigs.tile([C, HW], F32)
    nc.sync.dma_start(x_sb[:], x_d[:])

    q4 = bigs.tile([128, HW], BF16)
    k4 = bigs.tile([128, HW], BF16)
    wt8 = bigs.tile([128, MT * WP], FP8)   # per m-tile [vT | ones | pad]
    xct = bigs.tile([128, MT * C], F32)    # xcT, m-tile-major
    x_bf = bigs.tile([C, HW], BF16)

    # ones column of wt8 (wvc copies below only write cols 0..63)
    nc.vector.memset(
        wt8[:].rearrange("p (t c) -> p t c", c=WP)[:, :, 64:65], 1.0
    )

    # x in bf16 feeds the q4/k4/bn1 matmuls at full PE rate.
    nc.vector.tensor_copy(x_bf[:], x_sb[:])

    # ---- q4 / k4: replicated q,k via stacked-weight 1x1 convs ----
    # k4 first (block 0 needs all of k4 but only q4's first chunk).
    def qk_group(wT, dst, chunks, pool):
        ps = pool.tile([128, 2 * BLK], F32, tag="s", name="qkps")
        for i, cch in enumerate(chunks):
            nc.tensor.matmul(
                ps[:, i * BLK : (i + 1) * BLK],
                wT[:],
                x_bf[:, cch * BLK : (cch + 1) * BLK],
                start=True,
                stop=True,
            )
        w = len(chunks) * BLK
        lo = chunks[0] * BLK
        nc.vector.tensor_copy(dst[:, lo : lo + w], ps[:, :w])

    qk_group(wk4T, k4, [0, 1], spoolA)
    qk_group(wq4T, q4, [0, 1], spoolB)
    qk_group(wk4T, k4, [2, 3], spoolA)
    qk_group(wk4T, k4, [4, 5], spoolB)
    qk_group(wk4T, k4, [6, 7], spoolA)
    qk_group(wq4T, q4, [2, 3], spoolB)
    qk_group(wq4T, q4, [4, 5], spoolA)
    qk_group(wq4T, q4, [6, 7], spoolB)

    state = {}  # EC tile, allocated at block 1 start (vpool slot timing)

    def wvc_group(base, size):
        """xcT and WT (=[vT|ones]) production for one m-tile group."""
        ps_w = ppool.tile([128, BLK], F32, tag="p", name="wvcps")
        for j in range(size):
            m = base + j
            nc.tensor.matmul(
                ps_w[:, j * 128 : (j + 1) * 128],
                x_sb[:, m * 128 : (m + 1) * 128],
                wvc[:],
                start=True,
                stop=True,
            )
        src = ps_w[:, : size * 128].rearrange("p (j c) -> p j c", c=128)
        wt_dst = wt8[:, base * WP : (base + size) * WP].rearrange(
            "p (j c) -> p j c", c=WP
        )
        with nc.allow_low_precision(reason="v in fp8 for DoubleRow acc"):
            nc.vector.tensor_copy(wt_dst[:, :, 0:C], src[:, :, 0:C])
        xct_dst = xct[:, base * C : (base + size) * C].rearrange(
            "p (j c) -> p j c", c=C
        )
        nc.vector.tensor_copy(xct_dst, src[:, :, C : 2 * C])

    def ec_group(base, size):
        EC = state["EC"]
        for j in range(size):
            m = base + j
            nc.tensor.matmul(
                EC[0:C, 0:C],
                xct[:, m * C : (m + 1) * C],
                xct[:, m * C : (m + 1) * C],
                start=(m == 0),
                stop=(m == MT - 1),
            )

    # ---- per-block state for split epilogues ----
    vaccs = [None] * NB
    sam = [None] * NB   # sam65 [65, BLK] f32: rows 0..63 unnorm out_s, 64 = Z
    rzs = [None] * NB   # rz [65, BLK] bf16: row 64 = 1/Z
    M1T_sb = campool.tile([C, C], BF16)

    def epilogue_a(nb):
        """At block end: evacuate vacc, start the (slow, off-path) recip."""
        aux = sampool.tile([C + 1, BLK], F32, tag="aux", name="aux")
        nc.vector.tensor_copy(aux[:], vaccs[nb][0 : C + 1, :])
        rzb = sampool.tile([C + 1, BLK], BF16, tag="rz", name="rzb")
        with nc.allow_low_precision(reason="1/Z in bf16: 0.4% on the SAM term"):
            nc.vector.reciprocal(rzb[C : C + 1, :], aux[C : C + 1, :])
        sam[nb] = aux
        rzs[nb] = rzb

    def epilogue_b(nb):
        """Broadcast 1/Z, scale, bottleneck conv, residual add, DMA out."""
        ncol = slice(nb * BLK, (nb + 1) * BLK)
        bc = ppool.tile([128, BLK], F32, tag="p", name="bc")
        nc.tensor.matmul(
            bc[0:C, :],
            ones_r[C : C + 1, 0:C],
            rzs[nb][C : C + 1, :],
            start=True,
            stop=True,
            tile_position=(C, 0),
        )
        sam_sc = sampool.tile([C, BLK], BF16, tag="sc", name="sam_sc")
        nc.vector.tensor_mul(sam_sc[:], sam[nb][0:C, :], bc[0:C, :])
        bn = ppool.tile([128, BLK], F32, tag="p", name="bn")
        nc.tensor.matmul(
            bn[0:C, :], M1T_sb[:], x_bf[:, ncol], start=True, stop=False
        )
        nc.tensor.matmul(
            bn[0:C, :], wbn2T[:], sam_sc[:], start=False, stop=True
        )
        o_t = sampool.tile([C, BLK], F32, tag="ot", name="o_t")
        nc.vector.tensor_add(o_t[:], x_sb[:, ncol], bn[0:C, :])
        nc.sync.dma_start(out_d[:, ncol], o_t[:])

    # ---- main SAM loop over 8 n-blocks ----
    for nb in range(NB):
        ncol = slice(nb * BLK, (nb + 1) * BLK)
        if nb == 1:
            # EC takes the vpool slot vacated by the rotation; its last
            # readers (CAM softmax, end of block 1) finish before vacc(2)
            # re-claims the slot at block 2.
            state["EC"] = vpool.tile([128, BLK], F32, tag="v", name="EC")
        vacc = vpool.tile([128, BLK], F32, tag="v", name="vacc")
        vaccs[nb] = vacc
        for g in range(NG):
            base = 2 * g
            pool = spoolA if g % 2 == 0 else spoolB
            s_t = pool.tile([128, 2 * BLK], F32, tag="s", name="s_t")
            for j in range(2):
                m = base + j
                r = 2 * (g % 2) + j   # row quadrant: even g -> 0,1; odd -> 2,3
                nc.tensor.matmul(
                    s_t[:, j * BLK : (j + 1) * BLK],
                    k4[32 * r : 32 * r + 32, m * 128 : (m + 1) * 128],
                    q4[32 * r : 32 * r + 32, ncol],
                    start=True,
                    stop=True,
                    tile_position=(32 * r, 0),
                )
            if nb == 0:
                wvc_group(base, 2)   # wt8/xcT for this group (acc needs wt8)
            if nb == 1:
                ec_group(base, 2)    # CAM energy, spread across block 1
            e_t = epool.tile([128, 2 * BLK], FP8, tag="e", name="e_t")
            with nc.allow_low_precision(reason="E in fp8: ~1e-4 on out"):
                nc.scalar.activation(e_t[:], s_t[:], Exp, bias=nlog64[:])
            # one DoubleRow matmul contracts the m-tile pair (K=256)
            lhsT = wt8[:, base * WP : (base + 2) * WP].rearrange(
                "p (two f) -> p two f", two=2
            )[:, :, 0:65]
            rhs = e_t[:].rearrange("p (two f) -> p two f", two=2)
            nc.tensor.matmul(
                vacc[0 : C + 1, :],
                lhsT,
                rhs,
                start=(g == 0),
                stop=(g == NG - 1),
                perf_mode=DR,
            )
            if nb >= 2 and g == 1:
                # epilogue from two blocks ago, slotted into PE gaps
                epilogue_b(nb - 2)

        epilogue_a(nb)
        if nb == 1:
            # CAM softmax -> attn_c -> M1T = (wbn1 @ attn_c).T
            EC = state["EC"]
            negmax = campool.tile([C, 1], F32)
            nc.vector.reduce_max(
                negmax[:], EC[0:C, 0:C], axis=mybir.AxisListType.X, negate=True
            )
            exp_c = campool.tile([C, C], F32)
            nc.scalar.activation(exp_c[:], EC[0:C, 0:C], Exp, bias=negmax[:])
            sum_c = campool.tile([C, 1], F32)
            nc.vector.reduce_sum(sum_c[:], exp_c[:], axis=mybir.AxisListType.X)
            rec_c = campool.tile([C, 1], F32)
            nc.vector.reciprocal(rec_c[:], sum_c[:])
            attn_c = campool.tile([C, C], F32)
            nc.vector.tensor_scalar_mul(attn_c[:], exp_c[:], rec_c[:])
            m1ps = ppool.tile([128, BLK], F32, tag="p", name="m1ps")
            nc.tensor.matmul(
                m1ps[0:C, 0:C], attn_c[:], wbn1T[:], start=True, stop=True
            )
            nc.vector.tensor_copy(M1T_sb[:], m1ps[0:C, 0:C])
    epilogue_b(NB - 2)
    epilogue_b(NB - 1)


def build_nc():
    nc = bacc.Bacc(
        "TRN2",
        target_bir_lowering=False,
        debug=False,
        enable_asserts=False,
        num_devices=8,
    )
    io = {}
    io["x"] = nc.dram_tensor("x", [C, HW], F32, kind="ExternalInput").ap()
    io["wq4T"] = nc.dram_tensor("wq4T", [C, 128], BF16, kind="ExternalInput").ap()
    io["wk4T"] = nc.dram_tensor("wk4T", [C, 128], BF16, kind="ExternalInput").ap()
    io["wvc"] = nc.dram_tensor("wvc", [C, 128], F32, kind="ExternalInput").ap()
    io["wbn1T"] = nc.dram_tensor("wbn1T", [C, C], F32, kind="ExternalInput").ap()
    io["wbn2T"] = nc.dram_tensor("wbn2T", [C, C], BF16, kind="ExternalInput").ap()
    io["ones64"] = nc.dram_tensor("ones64", [1, C], BF16, kind="ExternalInput").ap()
    io["out"] = nc.dram_tensor("out", [C, HW], F32, kind="ExternalOutput").ap()

    with tile.TileContext(nc) as tc:
        with ExitStack() as ctx:
            _build_kernel(ctx, tc, io)
    nc.compile()
    return nc


def make_in_maps(x, w_cam, w_q, w_k, w_v, w_bn):
    import ml_dtypes

    f = lambda a: np.ascontiguousarray(np.asarray(a, dtype=np.float32))
    fb = lambda a: np.ascontiguousarray(
        np.asarray(a, dtype=np.float32).astype(ml_dtypes.bfloat16)
    )
    base = {
        "wq4T": fb(np.concatenate([np.asarray(w_q).T] * 4, axis=1)),
        "wk4T": fb(np.concatenate([np.asarray(w_k).T] * 4, axis=1)),
        "wvc": f(np.concatenate([np.asarray(w_v).T, np.asarray(w_cam).T], axis=1)),
        "wbn1T": f(np.asarray(w_bn)[:, :C].T),
        "wbn2T": fb(np.asarray(w_bn)[:, C:].T),
        "ones64": fb(np.ones((1, C))),
    }
    x = np.asarray(x)
    return [dict(base, x=f(x[b].reshape(C, HW))) for b in range(8)]


_NC_CACHE = None


def kernel(x, w_cam, w_q, w_k, w_v, w_bn):
    global _NC_CACHE
    if _NC_CACHE is None:
        _NC_CACHE = build_nc()
    nc = _NC_CACHE
    in_maps = make_in_maps(x, w_cam, w_q, w_k, w_v, w_bn)
    res = run_bass_kernel_spmd(nc, in_maps, list(range(8)))
    out = np.stack([res.results[b]["out"].reshape(C, 64, 64) for b in range(8)])
    return out.astype(np.float32)


# revision 17
# speedup vs baseline: 1.3957x; 1.0832x over previous
"""Trainium2 Bass kernel for dual-attention block (CAM + SAM + bottleneck).

Contract: kernel(**inputs) takes FULL unsharded inputs
  x     [8, 64, 64, 64] f32
  w_cam [64, 64], w_q [32, 64], w_k [32, 64], w_v [64, 64], w_bn [64, 128]
and returns the full [8, 64, 64, 64] f32 output.

Sharding: data-parallel over batch across 8 NeuronCores (1 image each);
weights replicated. Per-core math (c=64 channels, n=m=4096 spatial):

  CAM: xcT = x.T @ w_cam.T ; Ec = xcT.T @ xcT;
       attn_c = softmax_rows(Ec); bn1 = (wbn1 @ attn_c) @ x   (folded M1)
  SAM: q4/k4 = (w stacked 4x) @ x  -> q,k replicated on 4 partition groups
       S[m,n] = sum_c k[c,m] q[c,n]  (row-tiled K=32 matmuls, 4-concurrent)
       E = exp(S - ln64) in fp8-e4m3  (max|S|=9.05 -> E'max 133 < 240;
           the 1/64 cancels between numerator and denominator)
       acc[c,n] = sum_m W[m,c] E[m,n]  with W = [v.T | ones] in fp8,
                  one DoubleRow matmul per m-tile PAIR (K=256 contraction)
                  -> rows 0..63 unnormalized out_s, row 64 = Z
  out = x + bn1 + (wbn2 @ acc[0:64]) * (1/Z)
        (per-n 1/Z broadcast to 64 partitions via a K=1 PE matmul)

Design (v4): ScalarE (ACT) is the bound -- 16.8M exp at 1/lane/cycle
~= 110us/core (143us with per-instr overhead).  The HAM clock gate
keeps a <100%-duty PE at 1.2 GHz, so the PE workload is cut (fp8
DoubleRow acc; bf16 wvc/ec) and S matmuls are emitted in GROUP PAIRS
so 4 K=32 matmuls run concurrently on disjoint row quadrants before
each full-array DR matmul -- cold-PE ~940ns/group < 1114ns exp pace.
Preamble: x DMA split across both HWDGE queues (sync+scalar), chunked
x_bf cast, q4/k4 PSUM evacuation alternating DVE/ACT -> first exp
~11us.  The 3.3us DVE reciprocal and the deferred epilogues are
slotted mid-block so they never head-of-line-block the PE/DVE FIFOs.
PSUM: spoolA(2) + spoolB(2) + vacc/EC(2) + ppool(2) = 8 banks.
"""

import sys
from contextlib import ExitStack

import numpy as np

if "/opt/trn_rl_repo" not in sys.path:
    sys.path.insert(0, "/opt/trn_rl_repo")

import concourse.bass as bass
import concourse.tile as tile
from concourse import bacc, mybir
from concourse.bass_utils import run_bass_kernel_spmd

F32 = mybir.dt.float32
BF16 = mybir.dt.bfloat16
FP8 = mybir.dt.float8e4

C = 64          # channels
HW = 4096       # 64*64 spatial
NB = 8          # number of 512-wide n blocks
BLK = 512
MT = 32         # m tiles of 128
NG = 16         # groups of 2 m-tiles per n-block
WP = 80         # wt8 per-m-tile stride (65 used; 80 for DoubleRow step%16==0)
NLOG64 = -4.1588830833596715

Exp = mybir.ActivationFunctionType.Exp
DR = mybir.MatmulPerfMode.DoubleRow


def _build_kernel(ctx: ExitStack, tc: tile.TileContext, io: dict):
    nc = tc.nc
    x_d = io["x"]
    out_d = io["out"]

    consts = ctx.enter_context(tc.tile_pool(name="consts", bufs=1))
    bigs = ctx.enter_context(tc.tile_pool(name="bigs", bufs=1))
    epool = ctx.enter_context(tc.tile_pool(name="epool", bufs=3))
    campool = ctx.enter_context(tc.tile_pool(name="campool", bufs=1))
    sampool = ctx.enter_context(tc.tile_pool(name="sampool", bufs=2))
    spoolA = ctx.enter_context(
        tc.tile_pool(name="spoolA", bufs=1, space=bass.MemorySpace.PSUM)
    )
    spoolB = ctx.enter_context(
        tc.tile_pool(name="spoolB", bufs=1, space=bass.MemorySpace.PSUM)
    )
    vpool = ctx.enter_context(
        tc.tile_pool(name="vpool", bufs=2, space=bass.MemorySpace.PSUM)
    )
    ppool = ctx.enter_context(
        tc.tile_pool(name="ppool", bufs=2, space=bass.MemorySpace.PSUM)
    )

    # ---- x DMA first, split across both HWDGE queues ----
    x_sb = bigs.tile([C, HW], F32)
    nc.sync.dma_start(x_sb[:, 0 : HW // 2], x_d[:, 0 : HW // 2])
    nc.scalar.dma_start(x_sb[:, HW // 2 :], x_d[:, HW // 2 :])

    # ---- constants ----
    wq4T = consts.tile([C, 128], BF16)    # (w_q stacked 4x).T
    wk4T = consts.tile([C, 128], BF16)
    wvc = consts.tile([C, 128], BF16)     # [v.T | w_cam.T]
    wbn1T = consts.tile([C, C], F32)
    wbn2T = consts.tile([C, C], BF16)
    ones_r = consts.tile([128, C], BF16)  # row 64 holds ones[1, 64]
    zb = consts.tile([128, 1], F32)
    nlog64 = consts.tile([128, 1], F32)   # exp bias: E'=E/64 fits fp8e4 max 240
    dummy = consts.tile([128, 1], F32)

    nc.sync.dma_start(wk4T[:], io["wk4T"][:])
    nc.sync.dma_start(wq4T[:], io["wq4T"][:])
    nc.scalar.dma_start(wvc[:], io["wvc"][:])
    nc.scalar.dma_start(wbn1T[:], io["wbn1T"][:])
    nc.scalar.dma_start(wbn2T[:], io["wbn2T"][:])
    nc.scalar.dma_start(ones_r[C : C + 1, :], io["ones64"][:])
    nc.vector.memset(zb[:], 0.0)
    nc.vector.memset(nlog64[:], NLOG64)
    # Trigger the exp ACT-table load at t~0 (overlaps the x DMA) instead of
    # in front of the first real exp.
    nc.scalar.activation(dummy[:], zb[:], Exp, bias=zb[:])

    q4 = bigs.tile([128, HW], BF16)
    k4 = bigs.tile([128, HW], BF16)
    wt8 = bigs.tile([128, MT * WP], FP8)   # per m-tile [vT | ones | pad]
    xct = bigs.tile([128, MT * C], BF16)   # xcT, m-tile-major
    x_bf = bigs.tile([C, HW], BF16)

    # ones column of wt8 (wvc copies below only write cols 0..63)
    nc.vector.memset(
        wt8[:].rearrange("p (t c) -> p t c", c=WP)[:, :, 64:65], 1.0
    )

    # x in bf16 feeds the q4/k4/wvc/bn1 matmuls at full PE rate (2 chunks so
    # the first q/k matmuls start as soon as the first x half lands).
    nc.vector.tensor_copy(x_bf[:, 0 : HW // 2], x_sb[:, 0 : HW // 2])
    nc.vector.tensor_copy(x_bf[:, HW // 2 :], x_sb[:, HW // 2 :])

    # ---- q4 / k4: replicated q,k via stacked-weight 1x1 convs ----
    # k4 first (block 0 needs all of k4 but only q4's first chunk);
    # PSUM evacuation alternates DVE / ACT (ACT is idle in the preamble).
    def qk_group(wT, dst, chunks, pool, on_scalar):
        ps = pool.tile([128, 2 * BLK], F32, tag="s", name="qkps")
        for i, cch in enumerate(chunks):
            nc.tensor.matmul(
                ps[:, i * BLK : (i + 1) * BLK],
                wT[:],
                x_bf[:, cch * BLK : (cch + 1) * BLK],
                start=True,
                stop=True,
            )
        w = len(chunks) * BLK
        lo = chunks[0] * BLK
        if on_scalar:
            nc.scalar.copy(dst[:, lo : lo + w], ps[:, :w])
        else:
            nc.vector.tensor_copy(dst[:, lo : lo + w], ps[:, :w])

    qk_group(wk4T, k4, [0, 1], spoolA, False)
    qk_group(wq4T, q4, [0, 1], spoolB, True)
    qk_group(wk4T, k4, [2, 3], spoolA, False)
    qk_group(wk4T, k4, [4, 5], spoolB, True)
    qk_group(wk4T, k4, [6, 7], spoolA, False)
    qk_group(wq4T, q4, [2, 3], spoolB, True)
    qk_group(wq4T, q4, [4, 5], spoolA, False)
    qk_group(wq4T, q4, [6, 7], spoolB, True)

    state = {}  # EC tile, allocated at block 1 start (vpool slot timing)

    def wvc_group(base, size):
        """xcT and WT (=[vT|ones]) production for one m-tile group."""
        ps_w = ppool.tile([128, BLK], F32, tag="p", name="wvcps")
        for j in range(size):
            m = base + j
            nc.tensor.matmul(
                ps_w[:, j * 128 : (j + 1) * 128],
                x_bf[:, m * 128 : (m + 1) * 128],
                wvc[:],
                start=True,
                stop=True,
            )
        src = ps_w[:, : size * 128].rearrange("p (j c) -> p j c", c=128)
        wt_dst = wt8[:, base * WP : (base + size) * WP].rearrange(
            "p (j c) -> p j c", c=WP
        )
        with nc.allow_low_precision(reason="v in fp8 for DoubleRow acc"):
            nc.vector.tensor_copy(wt_dst[:, :, 0:C], src[:, :, 0:C])
        xct_dst = xct[:, base * C : (base + size) * C].rearrange(
            "p (j c) -> p j c", c=C
        )
        with nc.allow_low_precision(reason="xcT in bf16 for cheap ec matmuls"):
            nc.vector.tensor_copy(xct_dst, src[:, :, C : 2 * C])

    def ec_group(base, size):
        EC = state["EC"]
        for j in range(size):
            m = base + j
            nc.tensor.matmul(
                EC[0:C, 0:C],
                xct[:, m * C : (m + 1) * C],
                xct[:, m * C : (m + 1) * C],
                start=(m == 0),
                stop=(m == MT - 1),
            )

    # ---- per-block state for split epilogues ----
    vaccs = [None] * NB
    sam = [None] * NB   # sam65 [65, BLK] f32: rows 0..63 unnorm out_s, 64 = Z
    rzs = [None] * NB   # rz [65, BLK] bf16: row 64 = 1/Z
    M1T_sb = campool.tile([C, C], BF16)

    def epilogue_a(nb):
        """At block end: evacuate vacc (recip is emitted separately)."""
        aux = sampool.tile([C + 1, BLK], F32, tag="aux", name="aux")
        nc.vector.tensor_copy(aux[:], vaccs[nb][0 : C + 1, :])
        sam[nb] = aux

    def emit_recip(nb):
        """1/Z for block nb; emitted at a quiet mid-block DVE slot."""
        rzb = sampool.tile([C + 1, BLK], BF16, tag="rz", name="rzb")
        with nc.allow_low_precision(reason="1/Z in bf16: 0.4% on the SAM term"):
            nc.vector.reciprocal(rzb[C : C + 1, :], sam[nb][C : C + 1, :])
        rzs[nb] = rzb

    def epilogue_b1(nb):
        """Broadcast 1/Z to 64 partitions and scale the SAM accumulator."""
        bc = ppool.tile([128, BLK], F32, tag="p", name="bc")
        nc.tensor.matmul(
            bc[0:C, :],
            ones_r[C : C + 1, 0:C],
            rzs[nb][C : C + 1, :],
            start=True,
            stop=True,
            tile_position=(C, 0),
        )
        sam_sc = sampool.tile([C, BLK], BF16, tag="sc", name="sam_sc")
        nc.vector.tensor_mul(sam_sc[:], sam[nb][0:C, :], bc[0:C, :])
        return sam_sc

    def epilogue_b2(nb, sam_sc):
        """Bottleneck conv, residual add, DMA out."""
        ncol = slice(nb * BLK, (nb + 1) * BLK)
        bn = ppool.tile([128, BLK], F32, tag="p", name="bn")
        nc.tensor.matmul(
            bn[0:C, :], M1T_sb[:], x_bf[:, ncol], start=True, stop=False
        )
        nc.tensor.matmul(
            bn[0:C, :], wbn2T[:], sam_sc[:], start=False, stop=True
        )
        o_t = sampool.tile([C, BLK], F32, tag="ot", name="o_t")
        nc.vector.tensor_add(o_t[:], x_sb[:, ncol], bn[0:C, :])
        nc.sync.dma_start(out_d[:, ncol], o_t[:])

    def cam_chain():
        """CAM softmax -> attn_c -> M1T = (wbn1 @ attn_c).T"""
        EC = state["EC"]
        negmax = campool.tile([C, 1], F32)
        nc.vector.reduce_max(
            negmax[:], EC[0:C, 0:C], axis=mybir.AxisListType.X, negate=True
        )
        exp_c = campool.tile([C, C], F32)
        nc.scalar.activation(exp_c[:], EC[0:C, 0:C], Exp, bias=negmax[:])
        sum_c = campool.tile([C, 1], F32)
        nc.vector.reduce_sum(sum_c[:], exp_c[:], axis=mybir.AxisListType.X)
        rec_c = campool.tile([C, 1], F32)
        nc.vector.reciprocal(rec_c[:], sum_c[:])
        attn_c = campool.tile([C, C], F32)
        nc.vector.tensor_scalar_mul(attn_c[:], exp_c[:], rec_c[:])
        m1ps = ppool.tile([128, BLK], F32, tag="p", name="m1ps")
        nc.tensor.matmul(
            m1ps[0:C, 0:C], attn_c[:], wbn1T[:], start=True, stop=True
        )
        nc.vector.tensor_copy(M1T_sb[:], m1ps[0:C, 0:C])

    # ---- main SAM loop over 8 n-blocks, groups emitted in PAIRS ----
    sc_pend = {}  # nb -> sam_sc awaiting epilogue_b2
    for nb in range(NB):
        ncol = slice(nb * BLK, (nb + 1) * BLK)
        if nb == 1:
            # EC takes a vpool rotation slot; its last readers (CAM softmax,
            # start of block 2) finish before vacc(2) re-claims the slot.
            state["EC"] = vpool.tile([128, BLK], F32, tag="v", name="EC")
        if nb == 2:
            # CAM chain first so vacc(2), which aliases EC's bank, only
            # waits on the (fast) softmax reads of EC.
            cam_chain()
        vacc = vpool.tile([128, BLK], F32, tag="v", name="vacc")
        vaccs[nb] = vacc
        for p in range(NG // 2):
            g0, g1 = 2 * p, 2 * p + 1
            s_ts = []
            for g in (g0, g1):
                pool = spoolA if g % 2 == 0 else spoolB
                s_t = pool.tile([128, 2 * BLK], F32, tag="s", name="s_t")
                s_ts.append(s_t)
                for j in range(2):
                    m = 2 * g + j
                    r = 2 * (g % 2) + j  # row quadrants 0,1 / 2,3
                    nc.tensor.matmul(
                        s_t[:, j * BLK : (j + 1) * BLK],
                        k4[32 * r : 32 * r + 32, m * 128 : (m + 1) * 128],
                        q4[32 * r : 32 * r + 32, ncol],
                        start=True,
                        stop=True,
                        tile_position=(32 * r, 0),
                    )
            if nb == 0:
                wvc_group(2 * g0, 2)
                wvc_group(2 * g1, 2)
            if nb == 1:
                ec_group(2 * g0, 2)
                ec_group(2 * g1, 2)
            e_ts = []
            for g, s_t in zip((g0, g1), s_ts):
                e_t = epool.tile([128, 2 * BLK], FP8, tag="e", name="e_t")
                e_ts.append(e_t)
                with nc.allow_low_precision(reason="E in fp8: ~1e-4 on out"):
                    nc.scalar.activation(e_t[:], s_t[:], Exp, bias=nlog64[:])
            for g, e_t in zip((g0, g1), e_ts):
                lhsT = wt8[:, 2 * g * WP : (2 * g + 2) * WP].rearrange(
                    "p (two f) -> p two f", two=2
                )[:, :, 0:65]
                rhs = e_t[:].rearrange("p (two f) -> p two f", two=2)
                nc.tensor.matmul(
                    vacc[0 : C + 1, :],
                    lhsT,
                    rhs,
                    start=(g == 0),
                    stop=(g == NG - 1),
                    perf_mode=DR,
                )
            # deferred work, slotted into quiet spots mid-block:
            if nb >= 2 and p == 1:
                sc_pend[nb - 2] = epilogue_b1(nb - 2)
            if nb >= 2 and p == 2:
                epilogue_b2(nb - 2, sc_pend.pop(nb - 2))
            if nb >= 1 and p == 4:
                emit_recip(nb - 1)

        epilogue_a(nb)
    # ---- tail: last two blocks' epilogues ----
    sc6 = epilogue_b1(NB - 2)
    epilogue_b2(NB - 2, sc6)
    emit_recip(NB - 1)
    sc7 = epilogue_b1(NB - 1)
    epilogue_b2(NB - 1, sc7)


def build_nc():
    nc = bacc.Bacc(
        "TRN2",
        target_bir_lowering=False,
        debug=False,
        enable_asserts=False,
        num_devices=8,
    )
    io = {}
    io["x"] = nc.dram_tensor("x", [C, HW], F32, kind="ExternalInput").ap()
    io["wq4T"] = nc.dram_tensor("wq4T", [C, 128], BF16, kind="ExternalInput").ap()
    io["wk4T"] = nc.dram_tensor("wk4T", [C, 128], BF16, kind="ExternalInput").ap()
    io["wvc"] = nc.dram_tensor("wvc", [C, 128], BF16, kind="ExternalInput").ap()
    io["wbn1T"] = nc.dram_tensor("wbn1T", [C, C], F32, kind="ExternalInput").ap()
    io["wbn2T"] = nc.dram_tensor("wbn2T", [C, C], BF16, kind="ExternalInput").ap()
    io["ones64"] = nc.dram_tensor("ones64", [1, C], BF16, kind="ExternalInput").ap()
    io["out"] = nc.dram_tensor("out", [C, HW], F32, kind="ExternalOutput").ap()

    with tile.TileContext(nc) as tc:
        with ExitStack() as ctx:
            _build_kernel(ctx, tc, io)
    nc.compile()
    return nc


def make_in_maps(x, w_cam, w_q, w_k, w_v, w_bn):
    import ml_dtypes

    f = lambda a: np.ascontiguousarray(np.asarray(a, dtype=np.float32))
    fb = lambda a: np.ascontiguousarray(
        np.asarray(a, dtype=np.float32).astype(ml_dtypes.bfloat16)
    )
    base = {
        "wq4T": fb(np.concatenate([np.asarray(w_q).T] * 4, axis=1)),
        "wk4T": fb(np.concatenate([np.asarray(w_k).T] * 4, axis=1)),
        "wvc": fb(np.concatenate([np.asarray(w_v).T, np.asarray(w_cam).T], axis=1)),
        "wbn1T": f(np.asarray(w_bn)[:, :C].T),
        "wbn2T": fb(np.asarray(w_bn)[:, C:].T),
        "ones64": fb(np.ones((1, C))),
    }
    x = np.asarray(x)
    return [dict(base, x=f(x[b].reshape(C, HW))) for b in range(8)]


_NC_CACHE = None


def kernel(x, w_cam, w_q, w_k, w_v, w_bn):
    global _NC_CACHE
    if _NC_CACHE is None:
        _NC_CACHE = build_nc()
    nc = _NC_CACHE
    in_maps = make_in_maps(x, w_cam, w_q, w_k, w_v, w_bn)
    res = run_bass_kernel_spmd(nc, in_maps, list(range(8)))
    out = np.stack([res.results[b]["out"].reshape(C, 64, 64) for b in range(8)])
    return out.astype(np.float32)
